# revision 9
# baseline (speedup 1.0000x reference)
"""HOIContactLoss on Trainium2 — banded KNN with host planning.

Both cham_x (per-smpl-vertex NN in the object cloud) and cham_y (per-object-
vertex NN in the smpl cloud) reduce to ONE device job type: 128 sorted
queries vs a banded window of candidate columns, whose row-min the device
returns.  Soundness of the bands: for each query, the candidate blocks within
distance u of the query along a sort axis are included, where u is a true
distance to an actual candidate (min over K nearest-in-axis candidates on 3
axes) — the NN can then never lie outside the band.  High-u outlier queries
are extracted into their own tiles so they don't widen everyone's window.

Jobs are width-classed and dealt width-sorted round-robin to the 8 cores so
one SPMD program serves all cores; only the packed bf16 feature streams
differ per core.  d2 is built by a K=13 lifted matmul (hi/lo bf16 splits of
coords and squared norms recover ~fp32 accuracy) into PSUM f32.  Per slot one
of two paths: A — scalar engine drains PSUM (relu) to f16 and the DVE
min-folds at 2x into a per-group strip cascade; R — a single DVE tensor_reduce
takes the row-min straight from PSUM.  Paths are balanced per group so the
scalar and vector engines stay evenly loaded.  The host scatters the per-slot
[128] mins back, applies contact-map weights, and averages.
"""
import numpy as np
import ml_dtypes

import concourse.bacc as bacc
import concourse.tile as tile
from concourse import mybir
from concourse.bass_utils import run_bass_kernel_spmd
from contextlib import ExitStack

F32, F16, BF16 = mybir.dt.float32, mybir.dt.float16, mybir.dt.bfloat16
AOP = mybir.AluOpType
ACTF = mybir.ActivationFunctionType

B, P1, P2, D = 16, 6890, 4000, 3
NC = 8
K_BOUND = 96
FRAC_OUT = 0.125
QUANT = 128
WMAX = 1024
GRP = 8                       # slots per cascade group
BIGV = 60000.0                # pad c2 value (fits f16)
CLASSES = (512, 256, 128)     # strip sizes, processed big-first
BLK = 64                      # candidate gather-block granularity
AXES = (2, 0, 1)              # candidate sort orders to choose from per tile

_cache = {}


# ----------------------------- host planning ----------------------------- #

def _bf16c(a):
    return a.astype(ml_dtypes.bfloat16)


def feat_query(p):
    """Query feature rows [13,N]: coords hi/lo twice + q2 hi/lo + ones."""
    p = np.ascontiguousarray(p, np.float32)
    h = _bf16c(p); l = _bf16c(p - h.astype(np.float32))
    q2 = (p * p).sum(-1)
    q2h = _bf16c(q2); q2l = _bf16c(q2 - q2h.astype(np.float32))
    o = np.ones(len(p), ml_dtypes.bfloat16)
    return np.stack([h[:, 0], h[:, 1], h[:, 2], l[:, 0], l[:, 1], l[:, 2],
                     h[:, 0], h[:, 1], h[:, 2], q2h, q2l, o, o])


def feat_cand(p):
    """Candidate feature rows [13,N] pairing feat_query (t = -2p)."""
    p = np.ascontiguousarray(p, np.float32)
    t = -2.0 * p
    th = _bf16c(t); tl = _bf16c(t - th.astype(np.float32))
    c2 = (p * p).sum(-1)
    c2h = _bf16c(c2); c2l = _bf16c(c2 - c2h.astype(np.float32))
    o = np.ones(len(p), ml_dtypes.bfloat16)
    return np.stack([th[:, 0], th[:, 1], th[:, 2], th[:, 0], th[:, 1], th[:, 2],
                     tl[:, 0], tl[:, 1], tl[:, 2], o, o, c2h, c2l])


def _bound(qs, cs_sorted, axis):
    """Per-query sound NN-distance upper bound vs candidates sorted on axis."""
    qz = qs[:, axis]; cz = cs_sorted[:, axis]
    pos = np.searchsorted(cz, qz)
    cand = np.clip(pos[:, None] + np.arange(-K_BOUND, K_BOUND)[None, :], 0,
                   len(cs_sorted) - 1)
    diffs = qs[:, None, :] - cs_sorted[cand]
    return np.sqrt((diffs * diffs).sum(-1).min(1))


def plan_side(qs, cs):
    """qs, cs z-sorted (axis 2). Returns (jobs, rows_order, orders) where
    jobs = [(tile, order_id, block_idx_array)] with blocks of BLK candidate
    rows in `orders[order_id]` candidate ordering."""
    Nq, Nc = len(qs), len(cs)
    perms = [np.argsort(cs[:, a], kind='stable') for a in AXES]
    us = [_bound(qs, cs[perms[i]], a) for i, a in enumerate(AXES)]
    u = np.minimum.reduce(us)
    thr = np.quantile(u, 1.0 - FRAC_OUT)
    main = np.where(u <= thr)[0]
    outl = np.where(u > thr)[0]
    outl = outl[np.argsort(qs[outl, 0], kind='stable')]
    rows_order = np.concatenate([main, outl])
    NT = (Nq + 127) // 128
    NB = (Nc + BLK - 1) // BLK
    czs = [cs[perms[i]][:, a] for i, a in enumerate(AXES)]
    jobs = []
    for t in range(NT):
        rows = rows_order[t * 128:(t + 1) * 128]
        best = None
        for oid, (cz, ax) in enumerate(zip(czs, AXES)):
            qa = qs[rows, ax]
            lo = np.searchsorted(cz, qa - u[rows]) // BLK
            hi = np.minimum(np.searchsorted(cz, qa + u[rows]), Nc - 1) // BLK
            mask = np.zeros(NB, bool)
            for l, h in zip(lo, hi):
                mask[l:h + 1] = True
            blocks = np.where(mask)[0]
            if best is None or len(blocks) < len(best[1]):
                best = (oid, blocks)
        oid, blocks = best
        mb = WMAX // BLK
        for o in range(0, len(blocks), mb):
            jobs.append((t, oid, blocks[o:o + mb]))
    return jobs, rows_order, tuple(perms)


def strip_class(w):
    s = 128
    while 2 * s < w:
        s *= 2
    return s


def build_plan(smpl_v, object_v, smpl_contact_maps, object_contact_maps, ns):
    """Returns (schedule, items).
    schedule: list of (s_class, slots); slot = dict(w, path, percore)."""
    items = []
    all_jobs = {s: [] for s in CLASSES}
    for b in range(B):
        n = int(ns[b])
        x = np.asarray(smpl_v[b], np.float32)
        y = np.asarray(object_v[b, :n], np.float32)
        xi = np.argsort(x[:, 2], kind='stable')
        yi = np.argsort(y[:, 2], kind='stable')
        xs = x[xi]; ys = y[yi]
        jx, rox, ordx = plan_side(xs, ys)
        jy, roy, ordy = plan_side(ys, xs)
        it = dict(n=n, ro=(rox, roy), orders=(ordx, ordy),
                  qf=(feat_query(xs), feat_query(ys)),
                  cf=(feat_cand(ys), feat_cand(xs)),
                  sm=np.asarray(smpl_contact_maps[b, :, 0], np.float32)[xi],
                  om=np.asarray(object_contact_maps[b, :n, 0], np.float32)[yi])
        items.append(it)
        for side, jobs in ((0, jx), (1, jy)):
            for (t, oid, blocks) in jobs:
                w = ((BLK * len(blocks) + 127) // 128) * 128
                all_jobs[strip_class(w)].append(
                    dict(item=b, side=side, tile=t, oid=oid, blocks=blocks, w=w))
    schedule = []
    for s in CLASSES:
        jl = sorted(all_jobs[s], key=lambda j: -j['w'])
        nslots = (len(jl) + NC - 1) // NC
        slots = []
        for si in range(nslots):
            grp = jl[si * NC:(si + 1) * NC]
            slots.append(dict(w=grp[0]['w'],
                              percore=[grp[c] if c < len(grp) else None
                                       for c in range(NC)]))
        dummy_w = s + QUANT if s > 128 else 256
        while len(slots) % GRP:
            slots.append(dict(w=dummy_w, percore=[None] * NC, dummy=True))
        if slots:
            schedule.append((s, slots))
    # Path assignment per GRP-group: A (act-drain + f16 fold cascade:
    # act ~0.97w+init, vec ~1.04s) vs R (direct DVE tensor_reduce from PSUM:
    # vec ~1.04w+init).  Within each group start all-A and flip narrowest
    # slots to R until act and vec group loads balance — interleaving the two
    # paths so neither engine gets long saturated stretches.
    for s, slots in schedule:
        for gi in range(0, len(slots), GRP):
            g = slots[gi:gi + GRP]
            for sl in g:
                sl['path'] = 'R' if sl.get('dummy') else 'A'
            act_g = sum(0.97 * sl['w'] + 190 for sl in g if sl['path'] == 'A')
            vec_g = sum(1.04 * s + 60 for sl in g if sl['path'] == 'A') + \
                sum(1.04 * sl['w'] + 230 for sl in g if sl['path'] == 'R')
            for sl in sorted([x for x in g if x['path'] == 'A'],
                             key=lambda x: x['w']):
                if act_g <= vec_g:
                    break
                sl['path'] = 'R'
                act_g -= 0.97 * sl['w'] + 190
                vec_g += 1.04 * sl['w'] + 230 - 1.04 * s - 60
            g.sort(key=lambda sl: 0 if sl['path'] == 'A' else 1)
            slots[gi:gi + GRP] = g
    return schedule, items


def pack_streams(schedule, items):
    """Per-core bf16 stream [13, TOT] with [13,128] query + [13,w] candidate
    feature blocks per slot (padding columns carry c2 = BIG)."""
    tot = sum(128 + sl['w'] for _, slots in schedule for sl in slots)
    streams = [np.zeros((13, tot), ml_dtypes.bfloat16) for _ in range(NC)]
    for st in streams:
        st[11, :] = ml_dtypes.bfloat16(BIGV)
    off = 0
    for s, slots in schedule:
        for sl in slots:
            w = sl['w']
            for c in range(NC):
                j = sl['percore'][c]
                if j is not None:
                    it = items[j['item']]
                    side = j['side']
                    ro = it['ro'][side]
                    rows = ro[j['tile'] * 128:(j['tile'] + 1) * 128]
                    q = it['qf'][side][:, rows]
                    streams[c][:, off:off + len(rows)] = q
                    if len(rows) < 128:
                        streams[c][:, off + len(rows):off + 128] = 0
                    cf = it['cf'][side]
                    order = it['orders'][side][j['oid']]
                    nc_avail = cf.shape[1]
                    idx = np.concatenate([
                        order[bi * BLK:min((bi + 1) * BLK, nc_avail)]
                        for bi in j['blocks']])
                    streams[c][:, off + 128:off + 128 + len(idx)] = cf[:, idx]
                    if len(idx) < w:
                        blk = streams[c][:, off + 128 + len(idx):off + 128 + w]
                        blk[:] = 0
                        blk[11, :] = ml_dtypes.bfloat16(BIGV)
                else:
                    streams[c][:, off:off + 128] = 0
            off += 128 + w
    assert off == tot
    return streams, tot


def combine(schedule, items, res):
    """res [NC, 128, NSLOT] f32 per-slot row-mins -> final scalar loss."""
    chams = [[np.full(P1, np.inf, np.float32),
              np.full(it['n'], np.inf, np.float32)] for it in items]
    si = 0
    for s, slots in schedule:
        for sl in slots:
            for c in range(NC):
                j = sl['percore'][c]
                if j is None:
                    continue
                it = items[j['item']]
                ro = it['ro'][j['side']]
                rows = ro[j['tile'] * 128:(j['tile'] + 1) * 128]
                np.minimum.at(chams[j['item']][j['side']], rows,
                              res[c, :len(rows), si])
            si += 1
    losses = []
    for bidx, it in enumerate(items):
        cx = np.maximum(chams[bidx][0], 0.0)
        cy = np.maximum(chams[bidx][1], 0.0)
        lx = float((it['sm'] * cx).sum()) / (float(it['sm'].sum()) + 1e-6)
        ly = float((it['om'] * cy).sum()) / (float(it['om'].sum()) + 1e-6)
        losses.append(lx + ly)
    return np.float32(np.mean(losses))


def num_slots(schedule):
    return sum(len(slots) for _, slots in schedule)


# ----------------------------- device program ---------------------------- #

def _build(struct, tot, nslot):
    """struct: list of (s_class, [(w, path), ...]) with len(slots) % GRP == 0."""
    nc = bacc.Bacc(None, target_bir_lowering=False)
    with tile.TileContext(nc) as tc:
        with ExitStack() as ctx:
            dram = ctx.enter_context(tc.tile_pool(name="dram", bufs=1, space="DRAM"))
            const = ctx.enter_context(tc.tile_pool(name="const", bufs=1))
            candp = ctx.enter_context(tc.tile_pool(name="cand", bufs=3))
            drp = ctx.enter_context(tc.tile_pool(name="dr", bufs=4))
            stripp = ctx.enter_context(tc.tile_pool(name="strip", bufs=3))

            stream_d = dram.tile([13, tot], BF16, kind="ExternalInput")
            res_d = dram.tile([128, nslot], F32, kind="ExternalOutput")
            resacc = const.tile([128, nslot], F32)

            off = 0
            slot = 0
            for s, slots in struct:
                with ExitStack() as cctx:
                    nbuf = max(2, min(8, (16384 // (2 * s * 4))))
                    psp = cctx.enter_context(
                        tc.tile_pool(name=f"ps{s}", bufs=nbuf, space="PSUM"))
                    for g0 in range(0, len(slots), GRP):
                        grp = slots[g0:g0 + GRP]
                        seg = sum(128 + w for w, _ in grp)
                        cand = candp.tile([13, seg], BF16, tag="cand",
                                          name=f"cand_{s}_{g0}")
                        nc.sync.dma_start(out=cand[:],
                                          in_=stream_d[:, off:off + seg])
                        strip = stripp.tile([128, GRP, s], F16, tag="strip",
                                            name=f"strip_{s}_{g0}")
                        co = 0
                        na = sum(1 for _, p in grp if p == 'A')
                        for k, (w, path) in enumerate(grp):
                            lhsT = cand[:, co:co + 128]
                            ps = psp.tile([128, 2 * s], F32, tag="ps",
                                          name=f"ps_{s}_{g0}_{k}")
                            for o in range(0, w, 512):
                                cw = min(512, w - o)
                                nc.tensor.matmul(
                                    ps[:, o:o + cw], lhsT,
                                    cand[:, co + 128 + o:co + 128 + o + cw],
                                    start=True, stop=True)
                            d = w - s
                            if path == 'A':
                                dr = drp.tile([128, 2 * s], F16, tag="dr",
                                              name=f"dr_{s}_{g0}_{k}")
                                nc.scalar.activation(out=dr[:, 0:w], in_=ps[:, 0:w],
                                                     func=ACTF.Relu)
                                if d > 0:
                                    nc.vector.tensor_tensor(
                                        strip[:, k, :], dr[:, 0:s], dr[:, w - s:w],
                                        op=AOP.min)
                                else:
                                    nc.vector.tensor_copy(out=strip[:, k, :],
                                                          in_=dr[:, 0:s])
                            else:
                                nc.vector.tensor_reduce(
                                    out=resacc[:, slot + k:slot + k + 1],
                                    in_=ps[:, 0:w],
                                    axis=mybir.AxisListType.X, op=AOP.min)
                            co += 128 + w
                        if na > 0:
                            t = s
                            while t > 16:
                                h = t // 2
                                nc.vector.tensor_tensor(
                                    strip[:, 0:na, 0:h], strip[:, 0:na, 0:h],
                                    strip[:, 0:na, h:t], op=AOP.min)
                                t = h
                            nc.vector.tensor_reduce(
                                out=resacc[:, slot:slot + na],
                                in_=strip[:, 0:na, 0:16],
                                axis=mybir.AxisListType.X, op=AOP.min)
                        off += seg
                        slot += len(grp)
            nc.sync.dma_start(out=res_d[:], in_=resacc[:])
            names = dict(stream=stream_d.name, res=res_d.name)
    nc.compile()
    return nc, names


def kernel(smpl_v, object_v, smpl_contact_maps, object_contact_maps, object_verts_n,
           trace=False):
    smpl_v = np.asarray(smpl_v, np.float32)
    object_v = np.asarray(object_v, np.float32)
    smpl_contact_maps = np.asarray(smpl_contact_maps, np.float32)
    object_contact_maps = np.asarray(object_contact_maps, np.float32)
    ns = np.asarray(object_verts_n).astype(np.int64)

    schedule, items = build_plan(smpl_v, object_v, smpl_contact_maps,
                                 object_contact_maps, ns)
    streams, tot = pack_streams(schedule, items)
    nslot = num_slots(schedule)

    struct = tuple((s, tuple((sl['w'], sl['path']) for sl in slots))
                   for s, slots in schedule)
    key = (struct, tot, nslot)
    global _cache
    if key not in _cache:
        _cache = {key: _build([(s, list(sl)) for s, sl in struct], tot, nslot)}
    nc, names = _cache[key]

    in_maps = [{names['stream']: np.asarray(streams[c])} for c in range(NC)]
    res = run_bass_kernel_spmd(nc, in_maps, core_ids=list(range(NC)),
                               trace=trace)
    out = np.stack([res.results[c][names['res']] for c in range(NC)])
    loss = combine(schedule, items, out)
    if trace:
        return loss, res
    return loss


# revision 11
# speedup vs baseline: 1.0104x; 1.0104x over previous
"""HOIContactLoss on Trainium2 — banded KNN with host planning.

Both cham_x (per-smpl-vertex NN in the object cloud) and cham_y (per-object-
vertex NN in the smpl cloud) reduce to ONE device job type: 128 sorted
queries vs a banded window of candidate columns, whose row-min the device
returns.  Soundness of the bands: for each query, the candidate blocks within
distance u of the query along a sort axis are included, where u is a true
distance to an actual candidate (min over K nearest-in-axis candidates on 3
axes) — the NN can then never lie outside the band.  High-u outlier queries
are extracted into their own tiles so they don't widen everyone's window.

Jobs are width-classed and dealt width-sorted round-robin to the 8 cores so
one SPMD program serves all cores; only the packed bf16 feature streams
differ per core.  d2 is built by a K=13 lifted matmul (hi/lo bf16 splits of
coords and squared norms recover ~fp32 accuracy) into PSUM f32.  Per slot one
of two paths: A — scalar engine drains PSUM (relu) to f16 and the DVE
min-folds at 2x into a per-group strip cascade; R — a single DVE tensor_reduce
takes the row-min straight from PSUM.  Paths are balanced per group so the
scalar and vector engines stay evenly loaded.  The host scatters the per-slot
[128] mins back, applies contact-map weights, and averages.
"""
import numpy as np
import ml_dtypes

import concourse.bacc as bacc
import concourse.tile as tile
from concourse import mybir
from concourse.bass_utils import run_bass_kernel_spmd
from contextlib import ExitStack

F32, F16, BF16 = mybir.dt.float32, mybir.dt.float16, mybir.dt.bfloat16
AOP = mybir.AluOpType
ACTF = mybir.ActivationFunctionType

B, P1, P2, D = 16, 6890, 4000, 3
NC = 8
K_BOUND = 96
FRAC_OUT = 0.125
QUANT = 128
WMAX = 1024
GRP = 8                       # slots per cascade group
BIGV = 60000.0                # pad c2 value (fits f16)
CLASSES = (512, 256, 128)     # strip sizes, processed big-first
BLK = 64                      # candidate gather-block granularity
AXES = (2, 0, 1)              # candidate sort orders to choose from per tile

_cache = {}


# ----------------------------- host planning ----------------------------- #

def _bf16c(a):
    return a.astype(ml_dtypes.bfloat16)


def feat_query(p):
    """Query feature rows [13,N]: coords hi/lo twice + q2 hi/lo + ones."""
    p = np.ascontiguousarray(p, np.float32)
    h = _bf16c(p); l = _bf16c(p - h.astype(np.float32))
    q2 = (p * p).sum(-1)
    q2h = _bf16c(q2); q2l = _bf16c(q2 - q2h.astype(np.float32))
    o = np.ones(len(p), ml_dtypes.bfloat16)
    return np.stack([h[:, 0], h[:, 1], h[:, 2], l[:, 0], l[:, 1], l[:, 2],
                     h[:, 0], h[:, 1], h[:, 2], q2h, q2l, o, o])


def feat_cand(p):
    """Candidate feature rows [13,N] pairing feat_query (t = -2p)."""
    p = np.ascontiguousarray(p, np.float32)
    t = -2.0 * p
    th = _bf16c(t); tl = _bf16c(t - th.astype(np.float32))
    c2 = (p * p).sum(-1)
    c2h = _bf16c(c2); c2l = _bf16c(c2 - c2h.astype(np.float32))
    o = np.ones(len(p), ml_dtypes.bfloat16)
    return np.stack([th[:, 0], th[:, 1], th[:, 2], th[:, 0], th[:, 1], th[:, 2],
                     tl[:, 0], tl[:, 1], tl[:, 2], o, o, c2h, c2l])


def _bound(qs, cs_sorted, axis):
    """Per-query sound NN-distance upper bound vs candidates sorted on axis."""
    qz = qs[:, axis]; cz = cs_sorted[:, axis]
    pos = np.searchsorted(cz, qz)
    cand = np.clip(pos[:, None] + np.arange(-K_BOUND, K_BOUND)[None, :], 0,
                   len(cs_sorted) - 1)
    diffs = qs[:, None, :] - cs_sorted[cand]
    return np.sqrt((diffs * diffs).sum(-1).min(1))


def plan_side(qs, cs):
    """qs, cs z-sorted (axis 2). Returns (jobs, rows_order, orders) where
    jobs = [(tile, order_id, block_idx_array)] with blocks of BLK candidate
    rows in `orders[order_id]` candidate ordering."""
    Nq, Nc = len(qs), len(cs)
    perms = [np.argsort(cs[:, a], kind='stable') for a in AXES]
    us = [_bound(qs, cs[perms[i]], a) for i, a in enumerate(AXES)]
    u = np.minimum.reduce(us)
    thr = np.quantile(u, 1.0 - FRAC_OUT)
    main = np.where(u <= thr)[0]
    outl = np.where(u > thr)[0]
    outl = outl[np.argsort(qs[outl, 0], kind='stable')]
    rows_order = np.concatenate([main, outl])
    NT = (Nq + 127) // 128
    NB = (Nc + BLK - 1) // BLK
    czs = [cs[perms[i]][:, a] for i, a in enumerate(AXES)]
    jobs = []
    for t in range(NT):
        rows = rows_order[t * 128:(t + 1) * 128]
        best = None
        for oid, (cz, ax) in enumerate(zip(czs, AXES)):
            qa = qs[rows, ax]
            lo = np.searchsorted(cz, qa - u[rows]) // BLK
            hi = np.minimum(np.searchsorted(cz, qa + u[rows]), Nc - 1) // BLK
            mask = np.zeros(NB, bool)
            for l, h in zip(lo, hi):
                mask[l:h + 1] = True
            blocks = np.where(mask)[0]
            if best is None or len(blocks) < len(best[1]):
                best = (oid, blocks)
        oid, blocks = best
        mb = WMAX // BLK
        for o in range(0, len(blocks), mb):
            jobs.append((t, oid, blocks[o:o + mb]))
    return jobs, rows_order, tuple(perms)


def strip_class(w):
    s = 128
    while 2 * s < w:
        s *= 2
    return s


def build_plan(smpl_v, object_v, smpl_contact_maps, object_contact_maps, ns):
    """Returns (schedule, items).
    schedule: list of (s_class, slots); slot = dict(w, path, percore)."""
    items = []
    all_jobs = {s: [] for s in CLASSES}
    for b in range(B):
        n = int(ns[b])
        x = np.asarray(smpl_v[b], np.float32)
        y = np.asarray(object_v[b, :n], np.float32)
        xi = np.argsort(x[:, 2], kind='stable')
        yi = np.argsort(y[:, 2], kind='stable')
        xs = x[xi]; ys = y[yi]
        jx, rox, ordx = plan_side(xs, ys)
        jy, roy, ordy = plan_side(ys, xs)
        it = dict(n=n, ro=(rox, roy), orders=(ordx, ordy),
                  qf=(feat_query(xs), feat_query(ys)),
                  cf=(feat_cand(ys), feat_cand(xs)),
                  sm=np.asarray(smpl_contact_maps[b, :, 0], np.float32)[xi],
                  om=np.asarray(object_contact_maps[b, :n, 0], np.float32)[yi])
        items.append(it)
        for side, jobs in ((0, jx), (1, jy)):
            for (t, oid, blocks) in jobs:
                w = ((BLK * len(blocks) + 127) // 128) * 128
                all_jobs[strip_class(w)].append(
                    dict(item=b, side=side, tile=t, oid=oid, blocks=blocks, w=w))
    schedule = []
    for s in CLASSES:
        jl = sorted(all_jobs[s], key=lambda j: -j['w'])
        nslots = (len(jl) + NC - 1) // NC
        slots = []
        for si in range(nslots):
            grp = jl[si * NC:(si + 1) * NC]
            slots.append(dict(w=grp[0]['w'],
                              percore=[grp[c] if c < len(grp) else None
                                       for c in range(NC)]))
        dummy_w = s + QUANT if s > 128 else 256
        while len(slots) % GRP:
            slots.append(dict(w=dummy_w, percore=[None] * NC, dummy=True))
        if slots:
            schedule.append((s, slots))
    # Path assignment per GRP-group: A (act-drain + f16 fold cascade:
    # act ~0.97w+init, vec ~1.04s) vs R (direct DVE tensor_reduce from PSUM:
    # vec ~1.04w+init).  Within each group start all-A and flip narrowest
    # slots to R until act and vec group loads balance — interleaving the two
    # paths so neither engine gets long saturated stretches.
    for s, slots in schedule:
        for gi in range(0, len(slots), GRP):
            g = slots[gi:gi + GRP]
            for sl in g:
                sl['path'] = 'R' if sl.get('dummy') else 'A'
            act_g = sum(0.97 * sl['w'] + 190 for sl in g if sl['path'] == 'A')
            vec_g = sum(1.04 * s + 60 for sl in g if sl['path'] == 'A') + \
                sum(1.04 * sl['w'] + 230 for sl in g if sl['path'] == 'R')
            for sl in sorted([x for x in g if x['path'] == 'A'],
                             key=lambda x: x['w']):
                if act_g <= vec_g:
                    break
                sl['path'] = 'R'
                act_g -= 0.97 * sl['w'] + 190
                vec_g += 1.04 * sl['w'] + 230 - 1.04 * s - 60
            g.sort(key=lambda sl: 0 if sl['path'] == 'A' else 1)
            slots[gi:gi + GRP] = g
    return schedule, items


def pack_streams(schedule, items):
    """Per-core bf16 stream [13, TOT] with [13,128] query + [13,w] candidate
    feature blocks per slot (padding columns carry c2 = BIG)."""
    tot = sum(128 + sl['w'] for _, slots in schedule for sl in slots)
    streams = [np.zeros((13, tot), ml_dtypes.bfloat16) for _ in range(NC)]
    for st in streams:
        st[11, :] = ml_dtypes.bfloat16(BIGV)
    off = 0
    for s, slots in schedule:
        for sl in slots:
            w = sl['w']
            for c in range(NC):
                j = sl['percore'][c]
                if j is not None:
                    it = items[j['item']]
                    side = j['side']
                    ro = it['ro'][side]
                    rows = ro[j['tile'] * 128:(j['tile'] + 1) * 128]
                    q = it['qf'][side][:, rows]
                    streams[c][:, off:off + len(rows)] = q
                    if len(rows) < 128:
                        streams[c][:, off + len(rows):off + 128] = 0
                    cf = it['cf'][side]
                    order = it['orders'][side][j['oid']]
                    nc_avail = cf.shape[1]
                    idx = np.concatenate([
                        order[bi * BLK:min((bi + 1) * BLK, nc_avail)]
                        for bi in j['blocks']])
                    streams[c][:, off + 128:off + 128 + len(idx)] = cf[:, idx]
                    if len(idx) < w:
                        blk = streams[c][:, off + 128 + len(idx):off + 128 + w]
                        blk[:] = 0
                        blk[11, :] = ml_dtypes.bfloat16(BIGV)
                else:
                    streams[c][:, off:off + 128] = 0
            off += 128 + w
    assert off == tot
    return streams, tot


def combine(schedule, items, res):
    """res [NC, 128, NSLOT] f32 per-slot row-mins -> final scalar loss."""
    chams = [[np.full(P1, np.inf, np.float32),
              np.full(it['n'], np.inf, np.float32)] for it in items]
    si = 0
    for s, slots in schedule:
        for sl in slots:
            for c in range(NC):
                j = sl['percore'][c]
                if j is None:
                    continue
                it = items[j['item']]
                ro = it['ro'][j['side']]
                rows = ro[j['tile'] * 128:(j['tile'] + 1) * 128]
                np.minimum.at(chams[j['item']][j['side']], rows,
                              res[c, :len(rows), si])
            si += 1
    losses = []
    for bidx, it in enumerate(items):
        cx = np.maximum(chams[bidx][0], 0.0)
        cy = np.maximum(chams[bidx][1], 0.0)
        lx = float((it['sm'] * cx).sum()) / (float(it['sm'].sum()) + 1e-6)
        ly = float((it['om'] * cy).sum()) / (float(it['om'].sum()) + 1e-6)
        losses.append(lx + ly)
    return np.float32(np.mean(losses))


def num_slots(schedule):
    return sum(len(slots) for _, slots in schedule)


# ----------------------------- device program ---------------------------- #

def _build(struct, tot, nslot):
    """struct: list of (s_class, [(w, path), ...]) with len(slots) % GRP == 0."""
    nc = bacc.Bacc(None, target_bir_lowering=False)
    with tile.TileContext(nc) as tc:
        with ExitStack() as ctx:
            dram = ctx.enter_context(tc.tile_pool(name="dram", bufs=1, space="DRAM"))
            const = ctx.enter_context(tc.tile_pool(name="const", bufs=1))
            candp = ctx.enter_context(tc.tile_pool(name="cand", bufs=3))
            drp = ctx.enter_context(tc.tile_pool(name="dr", bufs=4))
            stripp = ctx.enter_context(tc.tile_pool(name="strip", bufs=3))

            stream_d = dram.tile([13, tot], BF16, kind="ExternalInput")
            res_d = dram.tile([128, nslot], F32, kind="ExternalOutput")
            resacc = const.tile([128, nslot], F32)

            off = 0
            slot = 0
            for s, slots in struct:
                with ExitStack() as cctx:
                    nbuf = max(2, min(8, (16384 // (2 * s * 4))))
                    psp = cctx.enter_context(
                        tc.tile_pool(name=f"ps{s}", bufs=nbuf, space="PSUM"))
                    for g0 in range(0, len(slots), GRP):
                        grp = slots[g0:g0 + GRP]
                        seg = sum(128 + w for w, _ in grp)
                        cand = candp.tile([13, seg], BF16, tag="cand",
                                          name=f"cand_{s}_{g0}")
                        nc.sync.dma_start(out=cand[:],
                                          in_=stream_d[:, off:off + seg])
                        strip = stripp.tile([128, GRP, s], F16, tag="strip",
                                            name=f"strip_{s}_{g0}")
                        co = 0
                        na = sum(1 for _, p in grp if p == 'A')
                        for k, (w, path) in enumerate(grp):
                            lhsT = cand[:, co:co + 128]
                            ps = psp.tile([128, 2 * s], F32, tag="ps",
                                          name=f"ps_{s}_{g0}_{k}")
                            for o in range(0, w, 512):
                                cw = min(512, w - o)
                                nc.tensor.matmul(
                                    ps[:, o:o + cw], lhsT,
                                    cand[:, co + 128 + o:co + 128 + o + cw],
                                    start=True, stop=True)
                            d = w - s
                            if path == 'A':
                                dr = drp.tile([128, 2 * s], F16, tag="dr",
                                              name=f"dr_{s}_{g0}_{k}")
                                nc.scalar.activation(out=dr[:, 0:w], in_=ps[:, 0:w],
                                                     func=ACTF.Relu)
                                if d > 0:
                                    nc.vector.tensor_tensor(
                                        strip[:, k, :], dr[:, 0:s], dr[:, w - s:w],
                                        op=AOP.min)
                                else:
                                    nc.vector.tensor_copy(out=strip[:, k, :],
                                                          in_=dr[:, 0:s])
                            else:
                                nc.vector.tensor_reduce(
                                    out=resacc[:, slot + k:slot + k + 1],
                                    in_=ps[:, 0:w],
                                    axis=mybir.AxisListType.X, op=AOP.min)
                            co += 128 + w
                        if na > 0:
                            t = s
                            while t > 32:
                                h = t // 2
                                nc.vector.tensor_tensor(
                                    strip[:, 0:na, 0:h], strip[:, 0:na, 0:h],
                                    strip[:, 0:na, h:t], op=AOP.min)
                                t = h
                            nc.vector.tensor_reduce(
                                out=resacc[:, slot:slot + na],
                                in_=strip[:, 0:na, 0:32],
                                axis=mybir.AxisListType.X, op=AOP.min)
                        off += seg
                        slot += len(grp)
            nc.sync.dma_start(out=res_d[:], in_=resacc[:])
            names = dict(stream=stream_d.name, res=res_d.name)
    nc.compile()
    return nc, names


def kernel(smpl_v, object_v, smpl_contact_maps, object_contact_maps, object_verts_n,
           trace=False):
    smpl_v = np.asarray(smpl_v, np.float32)
    object_v = np.asarray(object_v, np.float32)
    smpl_contact_maps = np.asarray(smpl_contact_maps, np.float32)
    object_contact_maps = np.asarray(object_contact_maps, np.float32)
    ns = np.asarray(object_verts_n).astype(np.int64)

    schedule, items = build_plan(smpl_v, object_v, smpl_contact_maps,
                                 object_contact_maps, ns)
    streams, tot = pack_streams(schedule, items)
    nslot = num_slots(schedule)

    struct = tuple((s, tuple((sl['w'], sl['path']) for sl in slots))
                   for s, slots in schedule)
    key = (struct, tot, nslot)
    global _cache
    if key not in _cache:
        _cache = {key: _build([(s, list(sl)) for s, sl in struct], tot, nslot)}
    nc, names = _cache[key]

    in_maps = [{names['stream']: np.asarray(streams[c])} for c in range(NC)]
    res = run_bass_kernel_spmd(nc, in_maps, core_ids=list(range(NC)),
                               trace=trace)
    out = np.stack([res.results[c][names['res']] for c in range(NC)])
    loss = combine(schedule, items, out)
    if trace:
        return loss, res
    return loss


# revision 12
# speedup vs baseline: 1.0115x; 1.0011x over previous
"""HOIContactLoss on Trainium2 — banded KNN with host planning.

Both cham_x (per-smpl-vertex NN in the object cloud) and cham_y (per-object-
vertex NN in the smpl cloud) reduce to ONE device job type: 128 sorted
queries vs a banded window of candidate columns, whose row-min the device
returns.  Soundness of the bands: for each query, the candidate blocks within
distance u of the query along a sort axis are included, where u is a true
distance to an actual candidate (min over K nearest-in-axis candidates on 3
axes) — the NN can then never lie outside the band.  High-u outlier queries
are extracted into their own tiles so they don't widen everyone's window.

Jobs are width-classed and dealt width-sorted round-robin to the 8 cores so
one SPMD program serves all cores; only the packed bf16 feature streams
differ per core.  d2 is built by a K=13 lifted matmul (hi/lo bf16 splits of
coords and squared norms recover ~fp32 accuracy) into PSUM f32.  Per slot one
of two paths: A — scalar engine drains PSUM (relu) to f16 and the DVE
min-folds at 2x into a per-group strip cascade; R — a single DVE tensor_reduce
takes the row-min straight from PSUM.  Paths are balanced per group so the
scalar and vector engines stay evenly loaded.  The host scatters the per-slot
[128] mins back, applies contact-map weights, and averages.
"""
import numpy as np
import ml_dtypes

import concourse.bacc as bacc
import concourse.tile as tile
from concourse import mybir
from concourse.bass_utils import run_bass_kernel_spmd
from contextlib import ExitStack

F32, F16, BF16 = mybir.dt.float32, mybir.dt.float16, mybir.dt.bfloat16
AOP = mybir.AluOpType
ACTF = mybir.ActivationFunctionType

B, P1, P2, D = 16, 6890, 4000, 3
NC = 8
K_BOUND = 96
FRAC_OUT = 0.125
QUANT = 128
WMAX = 1024
GRP = 8                       # slots per cascade group
BIGV = 60000.0                # pad c2 value (fits f16)
CLASSES = (512, 256, 128)     # strip sizes, processed big-first
BLK = 64                      # candidate gather-block granularity
AXES = (2, 0, 1)              # candidate sort orders to choose from per tile

_cache = {}


# ----------------------------- host planning ----------------------------- #

def _bf16c(a):
    return a.astype(ml_dtypes.bfloat16)


def feat_query(p):
    """Query feature rows [13,N]: coords hi/lo twice + q2 hi/lo + ones."""
    p = np.ascontiguousarray(p, np.float32)
    h = _bf16c(p); l = _bf16c(p - h.astype(np.float32))
    q2 = (p * p).sum(-1)
    q2h = _bf16c(q2); q2l = _bf16c(q2 - q2h.astype(np.float32))
    o = np.ones(len(p), ml_dtypes.bfloat16)
    return np.stack([h[:, 0], h[:, 1], h[:, 2], l[:, 0], l[:, 1], l[:, 2],
                     h[:, 0], h[:, 1], h[:, 2], q2h, q2l, o, o])


def feat_cand(p):
    """Candidate feature rows [13,N] pairing feat_query (t = -2p)."""
    p = np.ascontiguousarray(p, np.float32)
    t = -2.0 * p
    th = _bf16c(t); tl = _bf16c(t - th.astype(np.float32))
    c2 = (p * p).sum(-1)
    c2h = _bf16c(c2); c2l = _bf16c(c2 - c2h.astype(np.float32))
    o = np.ones(len(p), ml_dtypes.bfloat16)
    return np.stack([th[:, 0], th[:, 1], th[:, 2], th[:, 0], th[:, 1], th[:, 2],
                     tl[:, 0], tl[:, 1], tl[:, 2], o, o, c2h, c2l])


def _bound(qs, cs_sorted, axis):
    """Per-query sound NN-distance upper bound vs candidates sorted on axis."""
    qz = qs[:, axis]; cz = cs_sorted[:, axis]
    pos = np.searchsorted(cz, qz)
    cand = np.clip(pos[:, None] + np.arange(-K_BOUND, K_BOUND)[None, :], 0,
                   len(cs_sorted) - 1)
    diffs = qs[:, None, :] - cs_sorted[cand]
    return np.sqrt((diffs * diffs).sum(-1).min(1))


def plan_side(qs, cs):
    """qs, cs z-sorted (axis 2). Returns (jobs, rows_order, orders) where
    jobs = [(tile, order_id, block_idx_array)] with blocks of BLK candidate
    rows in `orders[order_id]` candidate ordering."""
    Nq, Nc = len(qs), len(cs)
    perms = [np.argsort(cs[:, a], kind='stable') for a in AXES]
    us = [_bound(qs, cs[perms[i]], a) for i, a in enumerate(AXES)]
    u = np.minimum.reduce(us)
    thr = np.quantile(u, 1.0 - FRAC_OUT)
    main = np.where(u <= thr)[0]
    outl = np.where(u > thr)[0]
    outl = outl[np.argsort(qs[outl, 0], kind='stable')]
    rows_order = np.concatenate([main, outl])
    NT = (Nq + 127) // 128
    NB = (Nc + BLK - 1) // BLK
    czs = [cs[perms[i]][:, a] for i, a in enumerate(AXES)]
    jobs = []
    for t in range(NT):
        rows = rows_order[t * 128:(t + 1) * 128]
        best = None
        for oid, (cz, ax) in enumerate(zip(czs, AXES)):
            qa = qs[rows, ax]
            lo = np.searchsorted(cz, qa - u[rows]) // BLK
            hi = np.minimum(np.searchsorted(cz, qa + u[rows]), Nc - 1) // BLK
            mask = np.zeros(NB, bool)
            for l, h in zip(lo, hi):
                mask[l:h + 1] = True
            blocks = np.where(mask)[0]
            if best is None or len(blocks) < len(best[1]):
                best = (oid, blocks)
        oid, blocks = best
        mb = WMAX // BLK
        for o in range(0, len(blocks), mb):
            jobs.append((t, oid, blocks[o:o + mb]))
    return jobs, rows_order, tuple(perms)


def strip_class(w):
    s = 128
    while 2 * s < w:
        s *= 2
    return s


def build_plan(smpl_v, object_v, smpl_contact_maps, object_contact_maps, ns):
    """Returns (schedule, items).
    schedule: list of (s_class, slots); slot = dict(w, path, percore)."""
    items = []
    all_jobs = {s: [] for s in CLASSES}
    for b in range(B):
        n = int(ns[b])
        x = np.asarray(smpl_v[b], np.float32)
        y = np.asarray(object_v[b, :n], np.float32)
        xi = np.argsort(x[:, 2], kind='stable')
        yi = np.argsort(y[:, 2], kind='stable')
        xs = x[xi]; ys = y[yi]
        jx, rox, ordx = plan_side(xs, ys)
        jy, roy, ordy = plan_side(ys, xs)
        it = dict(n=n, ro=(rox, roy), orders=(ordx, ordy),
                  qf=(feat_query(xs), feat_query(ys)),
                  cf=(feat_cand(ys), feat_cand(xs)),
                  sm=np.asarray(smpl_contact_maps[b, :, 0], np.float32)[xi],
                  om=np.asarray(object_contact_maps[b, :n, 0], np.float32)[yi])
        items.append(it)
        for side, jobs in ((0, jx), (1, jy)):
            for (t, oid, blocks) in jobs:
                w = ((BLK * len(blocks) + 127) // 128) * 128
                all_jobs[strip_class(w)].append(
                    dict(item=b, side=side, tile=t, oid=oid, blocks=blocks, w=w))
    schedule = []
    for s in CLASSES:
        jl = sorted(all_jobs[s], key=lambda j: -j['w'])
        nslots = (len(jl) + NC - 1) // NC
        slots = []
        for si in range(nslots):
            grp = jl[si * NC:(si + 1) * NC]
            slots.append(dict(w=grp[0]['w'],
                              percore=[grp[c] if c < len(grp) else None
                                       for c in range(NC)]))
        dummy_w = s + QUANT if s > 128 else 256
        while len(slots) % GRP:
            slots.append(dict(w=dummy_w, percore=[None] * NC, dummy=True))
        if slots:
            schedule.append((s, slots))
    # Path assignment per GRP-group: A (act-drain + f16 fold cascade:
    # act ~0.97w+init, vec ~1.04s) vs R (direct DVE tensor_reduce from PSUM:
    # vec ~1.04w+init).  Within each group start all-A and flip narrowest
    # slots to R until act and vec group loads balance — interleaving the two
    # paths so neither engine gets long saturated stretches.
    for s, slots in schedule:
        for gi in range(0, len(slots), GRP):
            g = slots[gi:gi + GRP]
            for sl in g:
                sl['path'] = 'R' if sl.get('dummy') else 'A'
            act_g = sum(0.97 * sl['w'] + 190 for sl in g if sl['path'] == 'A')
            vec_g = sum(1.04 * s + 60 for sl in g if sl['path'] == 'A') + \
                sum(1.04 * sl['w'] + 230 for sl in g if sl['path'] == 'R')
            for sl in sorted([x for x in g if x['path'] == 'A'],
                             key=lambda x: x['w']):
                if act_g <= vec_g:
                    break
                sl['path'] = 'R'
                act_g -= 0.97 * sl['w'] + 190
                vec_g += 1.04 * sl['w'] + 230 - 1.04 * s - 60
            g.sort(key=lambda sl: 0 if sl['path'] == 'A' else 1)
            slots[gi:gi + GRP] = g
    return schedule, items


def pack_streams(schedule, items):
    """Per-core bf16 stream [13, TOT] with [13,128] query + [13,w] candidate
    feature blocks per slot (padding columns carry c2 = BIG)."""
    tot = sum(128 + sl['w'] for _, slots in schedule for sl in slots)
    streams = [np.zeros((13, tot), ml_dtypes.bfloat16) for _ in range(NC)]
    for st in streams:
        st[11, :] = ml_dtypes.bfloat16(BIGV)
    off = 0
    for s, slots in schedule:
        for sl in slots:
            w = sl['w']
            for c in range(NC):
                j = sl['percore'][c]
                if j is not None:
                    it = items[j['item']]
                    side = j['side']
                    ro = it['ro'][side]
                    rows = ro[j['tile'] * 128:(j['tile'] + 1) * 128]
                    q = it['qf'][side][:, rows]
                    streams[c][:, off:off + len(rows)] = q
                    if len(rows) < 128:
                        streams[c][:, off + len(rows):off + 128] = 0
                    cf = it['cf'][side]
                    order = it['orders'][side][j['oid']]
                    nc_avail = cf.shape[1]
                    idx = np.concatenate([
                        order[bi * BLK:min((bi + 1) * BLK, nc_avail)]
                        for bi in j['blocks']])
                    streams[c][:, off + 128:off + 128 + len(idx)] = cf[:, idx]
                    if len(idx) < w:
                        blk = streams[c][:, off + 128 + len(idx):off + 128 + w]
                        blk[:] = 0
                        blk[11, :] = ml_dtypes.bfloat16(BIGV)
                else:
                    streams[c][:, off:off + 128] = 0
            off += 128 + w
    assert off == tot
    return streams, tot


def combine(schedule, items, res):
    """res [NC, 128, NSLOT] f32 per-slot row-mins -> final scalar loss."""
    chams = [[np.full(P1, np.inf, np.float32),
              np.full(it['n'], np.inf, np.float32)] for it in items]
    si = 0
    for s, slots in schedule:
        for sl in slots:
            for c in range(NC):
                j = sl['percore'][c]
                if j is None:
                    continue
                it = items[j['item']]
                ro = it['ro'][j['side']]
                rows = ro[j['tile'] * 128:(j['tile'] + 1) * 128]
                np.minimum.at(chams[j['item']][j['side']], rows,
                              res[c, :len(rows), si])
            si += 1
    losses = []
    for bidx, it in enumerate(items):
        cx = np.maximum(chams[bidx][0], 0.0)
        cy = np.maximum(chams[bidx][1], 0.0)
        lx = float((it['sm'] * cx).sum()) / (float(it['sm'].sum()) + 1e-6)
        ly = float((it['om'] * cy).sum()) / (float(it['om'].sum()) + 1e-6)
        losses.append(lx + ly)
    return np.float32(np.mean(losses))


def num_slots(schedule):
    return sum(len(slots) for _, slots in schedule)


# ----------------------------- device program ---------------------------- #

def _build(struct, tot, nslot):
    """struct: list of (s_class, [(w, path), ...]) with len(slots) % GRP == 0."""
    nc = bacc.Bacc(None, target_bir_lowering=False)
    with tile.TileContext(nc) as tc:
        with ExitStack() as ctx:
            dram = ctx.enter_context(tc.tile_pool(name="dram", bufs=1, space="DRAM"))
            const = ctx.enter_context(tc.tile_pool(name="const", bufs=1))
            candp = ctx.enter_context(tc.tile_pool(name="cand", bufs=3))
            drp = ctx.enter_context(tc.tile_pool(name="dr", bufs=4))
            stripp = ctx.enter_context(tc.tile_pool(name="strip", bufs=3))

            stream_d = dram.tile([13, tot], BF16, kind="ExternalInput")
            res_d = dram.tile([128, nslot], F32, kind="ExternalOutput")
            resacc = const.tile([128, nslot], F32)

            off = 0
            slot = 0
            for s, slots in struct:
                with ExitStack() as cctx:
                    nbuf = max(2, min(8, (16384 // (2 * s * 4))))
                    psp = cctx.enter_context(
                        tc.tile_pool(name=f"ps{s}", bufs=nbuf, space="PSUM"))
                    for g0 in range(0, len(slots), GRP):
                        grp = slots[g0:g0 + GRP]
                        seg = sum(128 + w for w, _ in grp)
                        cand = candp.tile([13, seg], BF16, tag="cand",
                                          name=f"cand_{s}_{g0}")
                        nc.sync.dma_start(out=cand[:],
                                          in_=stream_d[:, off:off + seg])
                        strip = stripp.tile([128, GRP, s], F16, tag="strip",
                                            name=f"strip_{s}_{g0}")
                        na = sum(1 for _, p in grp if p == 'A')
                        # per-slot stream offsets follow schedule order
                        cos = []
                        co = 0
                        for w, _ in grp:
                            cos.append(co)
                            co += 128 + w
                        # emit A and R slots interleaved (spread by rank) so
                        # scalar and vector work alternates within the group
                        nr = len(grp) - na
                        def _pos(k):
                            if k < na:
                                return (k + 0.5) / max(na, 1)
                            return (k - na + 0.5) / max(nr, 1)
                        order = sorted(range(len(grp)), key=_pos)
                        for k in order:
                            w, path = grp[k]
                            co = cos[k]
                            lhsT = cand[:, co:co + 128]
                            ps = psp.tile([128, 2 * s], F32, tag="ps",
                                          name=f"ps_{s}_{g0}_{k}")
                            for o in range(0, w, 512):
                                cw = min(512, w - o)
                                nc.tensor.matmul(
                                    ps[:, o:o + cw], lhsT,
                                    cand[:, co + 128 + o:co + 128 + o + cw],
                                    start=True, stop=True)
                            d = w - s
                            if path == 'A':
                                dr = drp.tile([128, 2 * s], F16, tag="dr",
                                              name=f"dr_{s}_{g0}_{k}")
                                nc.scalar.activation(out=dr[:, 0:w], in_=ps[:, 0:w],
                                                     func=ACTF.Relu)
                                if d > 0:
                                    nc.vector.tensor_tensor(
                                        strip[:, k, :], dr[:, 0:s], dr[:, w - s:w],
                                        op=AOP.min)
                                else:
                                    nc.vector.tensor_copy(out=strip[:, k, :],
                                                          in_=dr[:, 0:s])
                            else:
                                nc.vector.tensor_reduce(
                                    out=resacc[:, slot + k:slot + k + 1],
                                    in_=ps[:, 0:w],
                                    axis=mybir.AxisListType.X, op=AOP.min)
                        if na > 0:
                            t = s
                            while t > 32:
                                h = t // 2
                                nc.vector.tensor_tensor(
                                    strip[:, 0:na, 0:h], strip[:, 0:na, 0:h],
                                    strip[:, 0:na, h:t], op=AOP.min)
                                t = h
                            nc.vector.tensor_reduce(
                                out=resacc[:, slot:slot + na],
                                in_=strip[:, 0:na, 0:32],
                                axis=mybir.AxisListType.X, op=AOP.min)
                        off += seg
                        slot += len(grp)
            nc.sync.dma_start(out=res_d[:], in_=resacc[:])
            names = dict(stream=stream_d.name, res=res_d.name)
    nc.compile()
    return nc, names


def kernel(smpl_v, object_v, smpl_contact_maps, object_contact_maps, object_verts_n,
           trace=False):
    smpl_v = np.asarray(smpl_v, np.float32)
    object_v = np.asarray(object_v, np.float32)
    smpl_contact_maps = np.asarray(smpl_contact_maps, np.float32)
    object_contact_maps = np.asarray(object_contact_maps, np.float32)
    ns = np.asarray(object_verts_n).astype(np.int64)

    schedule, items = build_plan(smpl_v, object_v, smpl_contact_maps,
                                 object_contact_maps, ns)
    streams, tot = pack_streams(schedule, items)
    nslot = num_slots(schedule)

    struct = tuple((s, tuple((sl['w'], sl['path']) for sl in slots))
                   for s, slots in schedule)
    key = (struct, tot, nslot)
    global _cache
    if key not in _cache:
        _cache = {key: _build([(s, list(sl)) for s, sl in struct], tot, nslot)}
    nc, names = _cache[key]

    in_maps = [{names['stream']: np.asarray(streams[c])} for c in range(NC)]
    res = run_bass_kernel_spmd(nc, in_maps, core_ids=list(range(NC)),
                               trace=trace)
    out = np.stack([res.results[c][names['res']] for c in range(NC)])
    loss = combine(schedule, items, out)
    if trace:
        return loss, res
    return loss
